# revision 33
# baseline (speedup 1.0000x reference)
"""Trainium2 Bass kernel for nn_Attention_18940805775470.

8-sample batch of a per-sample attention block (EfficientViT-style
cascaded-group-attention cell):
  qkv 1x1 conv + BN -> 8-head attention (kd=16, hd=32, n=1024 tokens)
  -> + depthwise 3x3 BN branch on v -> 1x1 proj + BN.

Distribution: data-parallel, one sample per NeuronCore (B=8 == 8 cores).
All BN folds are done host-side; device does bf16 matmuls with fp32 PSUM
accumulation.

The kernel is softmax-bound: 8 heads x 1024^2 scores must be exp'd
(8.4M elements/core).  A single engine cannot do that fast enough, so the
exp stream is SPLIT across two engines running in parallel:
  - ScalarE: exact ACTIVATE(Exp) straight out of PSUM (~1.35us / [128,1024])
  - VectorE: Schraudolph bit-trick exp in ONE tensor_scalar op:
      i16 = rne(S * 2^7/ln2 + 16250); bitcast(i16) ~= bf16(exp(S)) +-3.5%
    (verified on HW: the DVE fp32->int16 output convert rounds to nearest;
    softmax normalization cancels most of the bias -> end-to-end rel-max
    error contribution ~2.8e-3, well inside the 2e-2 gate).
Per jc, score tile a (heads 0,1) goes to ScalarE and tile b (heads 2,3)
to VectorE; a few b tiles are reassigned to ScalarE to balance VectorE's
other elementwise work (copies / normalize).

Other structure vs the v1 kernel:
  - pe branch folded into proj: out = proj@tmp + proj@pe (+bias), PSUM
    accumulation -> the xattn adds disappear and the conv deadline moves
    to the proj at the end of each ic-half (conv units spread over all 4
    groups instead of crammed into ic=0).
  - memsets on GpSimd, off the critical VectorE.
  - scores S2[j,i] = k^T q per head, head-pairs into [128,1024] PSUM tiles
    via tile_position row tiling; softmax without max-subtraction (|S|<9).
  - ON[d,i] = v0 @ E2 and sums s[i] = 1^T E2 via 4-way column tiling.
  - qk-pack/vT/v/depthwise-conv jobs drip-fed into the PE's exp-wait slack.
"""

import sys

sys.path.insert(0, "/opt/trn_rl_repo")

import numpy as np
import ml_dtypes

BF16 = ml_dtypes.bfloat16

DIM = 256
NH = 8
HD = 32
KD = 16
SCALE = KD ** -0.5
EPS = 1e-3
B = 8
N = 1024  # 32*32 tokens
NCORES = 8
NGRP = 2  # head groups of 4

# Schraudolph constants: exp(x) ~= bitcast_bf16(int16(x * 2^7/ln2 + C))
EXP_A = float(2.0 ** 7 / np.log(2.0))
EXP_C = 16250.0

_CACHE = {}


def _build_host_weights(qkv_w, qkv_g, qkv_b, qkv_m, qkv_v,
                        pe_w, pe_g, pe_b, pe_m, pe_v,
                        proj_w, proj_g, proj_b, proj_m, proj_v):
    """Fold BN into weights and build the device-layout arrays."""
    inv_qkv = qkv_g / np.sqrt(qkv_v + EPS)
    Wq_full = qkv_w * inv_qkv[:, None]          # [512, 256]
    bq_full = qkv_b - qkv_m * inv_qkv           # [512]

    inv_pe = pe_g / np.sqrt(pe_v + EPS)
    bpe = pe_b - pe_m * inv_pe                  # [256]
    wpe = pe_w[:, 0] * inv_pe[:, None, None]    # [256, 3, 3]

    inv_p = proj_g / np.sqrt(proj_v + EPS)
    Pw = proj_w * inv_p[:, None]                # [256, 256]
    bp = proj_b - proj_m * inv_p                # [256]

    # q/k packed weight tiles: [128, NGRP*2*128]; block (g, kc) holds
    # lhsT [cc, m] with m = 32c + t.
    wq = np.zeros((128, NGRP * 2 * 128), np.float32)
    wk = np.zeros((128, NGRP * 2 * 128), np.float32)
    bqp = np.zeros((128, NGRP), np.float32)
    for g in range(NGRP):
        for c in range(4):
            h = 4 * g + c
            for kc in range(2):
                col0 = (g * 2 + kc) * 128
                # q rows (scaled); t in [0,16)
                wq[:, col0 + 32 * c: col0 + 32 * c + KD] = \
                    SCALE * Wq_full[h * 64: h * 64 + KD,
                                    kc * 128:(kc + 1) * 128].T
                # k rows, packed at the strip base like q (device reads
                # rows 32c..32c+16 of the separate kp tile)
                wk[:, col0 + 32 * c: col0 + 32 * c + KD] = \
                    Wq_full[h * 64 + KD: h * 64 + 2 * KD,
                            kc * 128:(kc + 1) * 128].T
            bqp[32 * c: 32 * c + KD, g] = \
                SCALE * bq_full[h * 64: h * 64 + KD]

    # v weights, channel-major (c = h*32 + d), transposed for lhsT/rhs use.
    vrows = np.array([(o // HD) * 64 + 2 * KD + (o % HD) for o in range(DIM)])
    Wv = Wq_full[vrows]                         # [256, 256]
    bv = bq_full[vrows]                         # [256]
    wv = np.zeros((128, 2 * 256), np.float32)   # [cc, kc*256 + o]
    for kc in range(2):
        wv[:, kc * 256:(kc + 1) * 256] = Wv[:, kc * 128:(kc + 1) * 128].T

    # depthwise conv diag tiles: [128, 2*9*128]
    dg = np.zeros((128, 2 * 9 * 128), np.float32)
    idx = np.arange(128)
    for ct in range(2):
        for tap in range(9):
            dy, dx = tap // 3, tap % 3
            blk = (ct * 9 + tap) * 128
            dg[idx, blk + idx] = wpe[ct * 128 + idx, dy, dx]

    # proj lhsT tiles: [128, (kc*2 + oc)*128 + o]
    pp = np.zeros((128, 4 * 128), np.float32)
    for kc in range(2):
        for oc in range(2):
            pp[:, (kc * 2 + oc) * 128:(kc * 2 + oc + 1) * 128] = \
                Pw[oc * 128:(oc + 1) * 128, kc * 128:(kc + 1) * 128].T

    bias_final = bp + Pw @ (bpe + bv)           # [256]

    bias_mat = np.zeros((128, 8), np.float32)
    bias_mat[:, 0:2] = bqp
    bias_mat[:, 2] = bv[:128]
    bias_mat[:, 3] = bv[128:]
    bias_mat[:, 4] = bias_final[:128]
    bias_mat[:, 5] = bias_final[128:]

    return {
        "wq": wq.astype(BF16),
        "wk": wk.astype(BF16),
        "wv": wv.astype(BF16),
        "dg": dg.astype(BF16),
        "pp": pp.astype(BF16),
        "bias": bias_mat,
    }


def _build_module():
    import concourse.bass as bass
    import concourse.mybir as mybir
    import concourse.tile as tile
    from concourse import bacc

    fp32 = mybir.dt.float32
    bf16 = mybir.dt.bfloat16
    i16 = mybir.dt.int16
    AF = mybir.ActivationFunctionType
    ALU = mybir.AluOpType

    nc = bacc.Bacc("TRN2", target_bir_lowering=False, debug=False,
                   num_devices=NCORES)

    x_d = nc.dram_tensor("x_bf", [DIM, N], bf16, kind="ExternalInput")
    wq_d = nc.dram_tensor("wq", [128, NGRP * 2 * 128], bf16, kind="ExternalInput")
    wk_d = nc.dram_tensor("wk", [128, NGRP * 2 * 128], bf16, kind="ExternalInput")
    wv_d = nc.dram_tensor("wv", [128, 2 * 256], bf16, kind="ExternalInput")
    dg_d = nc.dram_tensor("dg", [128, 2 * 9 * 128], bf16, kind="ExternalInput")
    pp_d = nc.dram_tensor("pp", [128, 4 * 128], bf16, kind="ExternalInput")
    bias_d = nc.dram_tensor("bias", [128, 8], fp32, kind="ExternalInput")
    y_d = nc.dram_tensor("y", [DIM, N], fp32, kind="ExternalOutput")

    with tile.TileContext(nc) as tc:
        from contextlib import ExitStack
        with ExitStack() as ctx:
            const = ctx.enter_context(tc.tile_pool(name="const", bufs=1))

            # ---- load inputs/weights ----
            xb = []
            for kc in range(2):
                t = const.tile([128, N], bf16, tag=f"xb{kc}", name=f"xb{kc}")
                for hh in range(2):
                    nc.sync.dma_start(
                        t[:, hh * 512:(hh + 1) * 512],
                        x_d[kc * 128:(kc + 1) * 128, hh * 512:(hh + 1) * 512])
                xb.append(t)
            wq_sb = const.tile([128, NGRP * 2 * 128], bf16, tag="wq")
            nc.gpsimd.dma_start(wq_sb[:], wq_d[:])
            wk_sb = const.tile([128, NGRP * 2 * 128], bf16, tag="wk")
            nc.gpsimd.dma_start(wk_sb[:], wk_d[:])
            bias_sb = const.tile([128, 8], fp32, tag="bias")
            nc.gpsimd.dma_start(bias_sb[:], bias_d[:])
            wv_sb = const.tile([128, 2 * 256], bf16, tag="wv")
            nc.gpsimd.dma_start(wv_sb[:], wv_d[:])
            dg_sb = const.tile([128, 2 * 9 * 128], bf16, tag="dg")
            nc.gpsimd.dma_start(dg_sb[:], dg_d[:])
            pp_sb = const.tile([128, 4 * 128], bf16, tag="pp")
            nc.gpsimd.dma_start(pp_sb[:], pp_d[:])

            ones_sb = const.tile([128, 32], bf16, tag="ones")
            nc.gpsimd.memset(ones_sb[:], 1.0)

            # ---- persistent intermediate tiles ----
            qp_sb = [const.tile([128, N], bf16, tag=f"qp{g}", name=f"qp{g}") for g in range(NGRP)]
            kp_sb = [const.tile([128, N], bf16, tag=f"kp{g}", name=f"kp{g}") for g in range(NGRP)]
            vt_sb = [const.tile([128, DIM], bf16, tag=f"vt{pc}", name=f"vt{pc}") for pc in range(8)]
            vpad = [const.tile([128, 34 * 34], bf16, tag=f"vpad{ct}", name=f"vpad{ct}") for ct in range(2)]
            pe_sb = [const.tile([128, N], bf16, tag=f"pe{ct}", name=f"pe{ct}") for ct in range(2)]
            tmp_sb = [const.tile([128, N], bf16, tag=f"tmp{ct}", name=f"tmp{ct}") for ct in range(2)]
            out_sb = [const.tile([128, N], fp32, tag=f"out{oc}", name=f"out{oc}") for oc in range(2)]

            for ct in range(2):
                nc.gpsimd.memset(vpad[ct][:], 0.0)

            # ====== unified pipeline: prologue jobs drip into attention ======
            # PSUM: scores 3x[128,1024] (6 banks) + on (1) + s (1) = 8.
            # All prologue-style matmul jobs (qk-pack, vT, v, depthwise conv)
            # borrow a scores slot briefly (tag "sc"), and are drip-fed into
            # the attention loop so the scores->exp stream starts ~immediately
            # and the PE fills its exp-wait slack with them.
            with tc.tile_pool(name="scA", bufs=2, space="PSUM") as scA, \
                 tc.tile_pool(name="scB", bufs=2, space="PSUM") as scB, \
                 tc.tile_pool(name="onps", bufs=1, space="PSUM") as onps, \
                 tc.tile_pool(name="sps", bufs=1, space="PSUM") as sps, \
                 tc.tile_pool(name="e2", bufs=8) as e2p, \
                 tc.tile_pool(name="e2q", bufs=16) as e2q, \
                 tc.tile_pool(name="nrm", bufs=4) as nrm:

                def mm_ksplit(out, lhsT, rhs, first, last):
                    nc.tensor.matmul(out, lhsT, rhs, start=first, stop=last)

                def q_job(g, nc2):
                    sl = slice(nc2 * 512, (nc2 + 1) * 512)
                    pq = scB.tile([128, 512], fp32, tag="scB", name="pq")
                    for kc in range(2):
                        col = (g * 2 + kc) * 128
                        mm_ksplit(pq[:], wq_sb[:, col:col + 128],
                                  xb[kc][:, sl], kc == 0, kc == 1)
                    nc.vector.tensor_scalar_add(
                        qp_sb[g][:, sl], pq[:], bias_sb[:, g:g + 1])

                def k_job(g, nc2):
                    sl = slice(nc2 * 512, (nc2 + 1) * 512)
                    pk = scB.tile([128, 512], fp32, tag="scB", name="pk")
                    for kc in range(2):
                        col = (g * 2 + kc) * 128
                        mm_ksplit(pk[:], wk_sb[:, col:col + 128],
                                  xb[kc][:, sl], kc == 0, kc == 1)
                    nc.vector.tensor_copy(kp_sb[g][:, sl], pk[:])

                def vt_job(pc):
                    pvt = scB.tile([128, 256], fp32, tag="scB", name="pvt")
                    for kc in range(2):
                        mm_ksplit(pvt[:], xb[kc][:, pc * 128:(pc + 1) * 128],
                                  wv_sb[:, kc * 256:(kc + 1) * 256],
                                  kc == 0, kc == 1)
                    nc.vector.tensor_copy(vt_sb[pc][:], pvt[:])

                def v_job(ct, nc2):
                    vp3 = vpad[ct][:].rearrange("p (a b) -> p a b", a=34)
                    y0 = nc2 * 16
                    pv = scB.tile([128, 512], fp32, tag="scB", name="pv")
                    for kc in range(2):
                        mm_ksplit(
                            pv[:],
                            wv_sb[:, kc * 256 + ct * 128: kc * 256 + ct * 128 + 128],
                            xb[kc][:, nc2 * 512:(nc2 + 1) * 512],
                            kc == 0, kc == 1)
                    nc.vector.tensor_scalar_add(
                        vp3[:, 1 + y0:1 + y0 + 16, 1:33],
                        pv[:].rearrange("p (a b) -> p a b", b=32),
                        bias_sb[:, 2 + ct:3 + ct])

                pe_state = {}

                def pe_third(ct, nc2, t3):
                    vp3 = vpad[ct][:].rearrange("p (a b) -> p a b", a=34)
                    y0 = nc2 * 16
                    if t3 == 0:
                        pe_state[(ct, nc2)] = scB.tile(
                            [128, 512], fp32, tag="scB", name="peps")
                    pp_ps = pe_state[(ct, nc2)]
                    for tap in range(3 * t3, 3 * t3 + 3):
                        dy, dx = tap // 3, tap % 3
                        blk = (ct * 9 + tap) * 128
                        for r in range(4):
                            nc.tensor.matmul(
                                pp_ps[32 * r:32 * r + 32, :],
                                dg_sb[32 * r:32 * r + 32,
                                      blk + 32 * r:blk + 32 * r + 32],
                                vp3[32 * r:32 * r + 32,
                                    y0 + dy:y0 + dy + 16, dx:dx + 32],
                                start=(tap == 0), stop=(tap == 8),
                                tile_position=(32 * r, 32 * r),
                                skip_group_check=True)
                    if t3 == 2:
                        nc.vector.tensor_copy(
                            pe_sb[ct][:, nc2 * 512:(nc2 + 1) * 512], pp_ps[:])

                # proj split per output half and decoupled from its +bias/DMA
                # so each piece is < one exp-duration of PE work and the
                # VectorE out-add never heads the queue before its pj is done.
                pj_state = {}

                def pj_pe(ic2, oc):
                    # pe-branch half of the proj accumulation; can run as
                    # soon as the conv outputs exist (before the group's
                    # normalize produces tmp).
                    isl2 = slice(ic2 * 512, (ic2 + 1) * 512)
                    pj = scB.tile([128, 512], fp32, tag="scB", name="pj")
                    pj_state[(ic2, oc)] = pj
                    for kc in range(2):
                        col = (kc * 2 + oc) * 128
                        mm_ksplit(pj[:], pp_sb[:, col:col + 128],
                                  pe_sb[kc][:, isl2], kc == 0, False)

                def pj_tmp0(ic2, oc):
                    isl2 = slice(ic2 * 512, (ic2 + 1) * 512)
                    pj = pj_state[(ic2, oc)]
                    mm_ksplit(pj[:], pp_sb[:, oc * 128:oc * 128 + 128],
                              tmp_sb[0][:, isl2], False, False)

                def pj_tmp1(ic2, oc):
                    isl2 = slice(ic2 * 512, (ic2 + 1) * 512)
                    pj = pj_state[(ic2, oc)]
                    col = (2 + oc) * 128
                    mm_ksplit(pj[:], pp_sb[:, col:col + 128],
                              tmp_sb[1][:, isl2], False, True)

                def pj_tmp(ic2, oc):
                    pj_tmp0(ic2, oc)
                    pj_tmp1(ic2, oc)

                def pj_job(ic2, oc):
                    pj_pe(ic2, oc)
                    pj_tmp(ic2, oc)

                def out_job(ic2, oc):
                    isl2 = slice(ic2 * 512, (ic2 + 1) * 512)
                    pj = pj_state.pop((ic2, oc))
                    nc.vector.tensor_scalar_add(
                        out_sb[oc][:, isl2], pj[:], bias_sb[:, 4 + oc:5 + oc])
                    nc.sync.dma_start(
                        y_d[oc * 128:(oc + 1) * 128, isl2],
                        out_sb[oc][:, isl2])

                # prologue: only what the first scores need.
                q_job(0, 0)
                k_job(0, 0)

                # drip schedule keyed by (gi, jc); conv unit (ct, nc2) is
                # only needed by proj(ic=nc2), i.e. by the end of group 1
                # (nc2=0) / group 3 (nc2=1), so conv spreads over all groups.
                # proj(ic=0) is deferred into group 2's slots; proj(ic=1)
                # runs inline after the last group.
                drip = {
                    (0, 0): [lambda: vt_job(0), lambda: vt_job(1)],
                    (0, 1): [lambda: vt_job(2), lambda: vt_job(3),
                             lambda: k_job(0, 1)],
                    (0, 2): [lambda: vt_job(4), lambda: v_job(0, 0)],
                    (0, 3): [lambda: vt_job(5), lambda: v_job(0, 1)],
                    (0, 4): [lambda: vt_job(6), lambda: vt_job(7)],
                    (0, 5): [lambda: pe_third(0, 0, 0)],
                    (0, 6): [lambda: pe_third(0, 0, 1)],
                    (0, 7): [lambda: q_job(1, 0), lambda: k_job(1, 0)],
                    (1, 0): [lambda: pe_third(0, 0, 2), lambda: k_job(1, 1)],
                    (1, 1): [lambda: v_job(1, 0), lambda: v_job(1, 1)],
                    (1, 2): [lambda: pe_third(1, 0, 0)],
                    (1, 3): [lambda: pe_third(1, 0, 1)],
                    (1, 4): [lambda: pe_third(1, 0, 2)],
                    (1, 5): [lambda: q_job(0, 1)],
                    (2, 0): [lambda: pj_job(0, 0)],
                    (2, 1): [lambda: out_job(0, 0), lambda: pe_third(0, 1, 0)],
                    (2, 2): [lambda: pe_third(0, 1, 1)],
                    (2, 3): [lambda: pe_third(0, 1, 2), lambda: pj_job(0, 1)],
                    (2, 4): [lambda: out_job(0, 1), lambda: q_job(1, 1)],
                    (3, 0): [lambda: pe_third(1, 1, 0)],
                    (3, 1): [lambda: pe_third(1, 1, 1)],
                    (3, 2): [lambda: pe_third(1, 1, 2)],
                    (3, 6): [lambda: pj_pe(1, 0), lambda: pj_tmp0(1, 0)],
                    (3, 7): [lambda: pj_pe(1, 1), lambda: pj_tmp0(1, 1)],
                }

                # exp-engine balance: tile a (heads 0,1) -> ScalarE; tile b
                # (heads 2,3) -> VectorE Schraudolph, except EXTRA_S where b
                # also goes to ScalarE (VectorE carries the other elementwise
                # work, so ScalarE takes ~60% of the exp tiles).
                EXTRA_S = {(0, 3), (1, 3), (2, 3), (3, 3)}

                def scores_one(gi, g, isl, jc, half):
                    """half 0 (heads 0,1): one [128,1024] tile from the
                    ScalarE pool, ACTIVATE(Exp).  half 1 (heads 2,3): two
                    [128,512] per-head tiles from the VectorE pool,
                    Schraudolph exp (ScalarE on EXTRA_S jc's).  Separate
                    pools keep each exp lane self-paced instead of
                    lockstepped through one rotation."""
                    if half == 0:
                        sc = scA.tile([128, 1024], fp32, tag="scA", name="sc")
                        for cc in range(2):
                            c = cc
                            nc.tensor.matmul(
                                sc[:, cc * 512:(cc + 1) * 512],
                                kp_sb[g][32 * c:32 * c + KD,
                                         jc * 128:(jc + 1) * 128],
                                qp_sb[g][32 * c:32 * c + KD, isl],
                                start=True, stop=True,
                                tile_position=(32 * c, 0))
                        e = e2p.tile([128, 1024], bf16, tag="e2", name="e2")
                        nc.scalar.activation(e[:], sc[:], AF.Exp)
                        return e
                    es = []
                    for cc in range(2):
                        c = 2 + cc
                        scb = scB.tile([128, 512], fp32, tag="scB", name="scb")
                        nc.tensor.matmul(
                            scb[:],
                            kp_sb[g][32 * c:32 * c + KD,
                                     jc * 128:(jc + 1) * 128],
                            qp_sb[g][32 * c:32 * c + KD, isl],
                            start=True, stop=True,
                            tile_position=(32 * c, 0))
                        eb = e2q.tile([128, 512], bf16, tag="e2q", name="e2q")
                        if (gi, jc) in EXTRA_S:
                            nc.scalar.activation(eb[:], scb[:], AF.Exp)
                        else:
                            nc.vector.tensor_scalar(
                                eb[:].bitcast(i16), scb[:], EXP_A, EXP_C,
                                ALU.mult, ALU.add)
                        es.append(eb)
                    return es

                # (ic, g) iteration order; the next group's jc0 scores are
                # prefetched before the previous group's vsums(7)+combine so
                # seams never stall the exp stream.
                groups = [(ic, g) for ic in range(2) for g in range(NGRP)]
                prefetched = None
                for gi, (ic, g) in enumerate(groups):
                    isl = slice(ic * 512, (ic + 1) * 512)
                    e2 = {}
                    if prefetched is not None:
                        e2[0] = prefetched
                    prefetched = None
                    on_ps = onps.tile([128, 512], fp32, tag="on", name="on")
                    s_ps = sps.tile([128, 512], fp32, tag="s", name="s")

                    def erhs(jc, c):
                        if c < 2:
                            return e2[jc][0][:, c * 512:(c + 1) * 512]
                        return e2[jc][1][c - 2][:]

                    def vsums(jc):
                        for c in range(4):
                            h = 4 * g + c
                            nc.tensor.matmul(
                                on_ps[32 * c:32 * c + 32, :],
                                vt_sb[jc][:, h * 32:(h + 1) * 32],
                                erhs(jc, c),
                                start=(jc == 0), stop=(jc == 7),
                                tile_position=(0, 32 * c),
                                skip_group_check=True)
                        for c in range(4):
                            nc.tensor.matmul(
                                s_ps[32 * c:32 * c + 32, :],
                                ones_sb[:],
                                erhs(jc, c),
                                start=(jc == 0), stop=(jc == 7),
                                tile_position=(0, 32 * c),
                                skip_group_check=True)

                    start = len(e2)
                    if start == 1:
                        for job in drip.get((gi, 0), []):
                            job()
                    for jc in range(start, 8):
                        e2[jc] = [scores_one(gi, g, isl, jc, 0),
                                  scores_one(gi, g, isl, jc, 1)]
                        if jc >= 1:
                            vsums(jc - 1)
                        for job in drip.get((gi, jc), []):
                            job()
                    if gi + 1 < len(groups):
                        nic, ng = groups[gi + 1]
                        nisl = slice(nic * 512, (nic + 1) * 512)
                        prefetched = [
                            scores_one(gi + 1, ng, nisl, 0, 0),
                            scores_one(gi + 1, ng, nisl, 0, 1)]
                    vsums(7)
                    rbc = nrm.tile([128, 512], fp32, tag="rbc", name="rbc")
                    nc.vector.reciprocal_approx_fast(rbc[:], s_ps[:])
                    nc.vector.tensor_mul(tmp_sb[g][:, isl], on_ps[:], rbc[:])

                # tail: proj(ic=1) could not be deferred into a later group.
                # pe-halves and the g=0 tmp-half ran in drip slots; only the
                # g=1 tmp matmuls sit behind the last normalize.
                pj_tmp1(1, 0)
                pj_tmp1(1, 1)
                out_job(1, 0)
                out_job(1, 1)

    nc.compile()
    return nc


def _get_module():
    if "nc" not in _CACHE:
        _CACHE["nc"] = _build_module()
    return _CACHE["nc"]


def kernel(x, qkv_w, qkv_g, qkv_b, qkv_m, qkv_v,
           pe_w, pe_g, pe_b, pe_m, pe_v,
           proj_w, proj_g, proj_b, proj_m, proj_v,
           _trace=False, _trace_kwargs=None):
    from concourse.bass_utils import run_bass_kernel_spmd

    w = _build_host_weights(
        np.asarray(qkv_w, np.float32), np.asarray(qkv_g, np.float32),
        np.asarray(qkv_b, np.float32), np.asarray(qkv_m, np.float32),
        np.asarray(qkv_v, np.float32),
        np.asarray(pe_w, np.float32), np.asarray(pe_g, np.float32),
        np.asarray(pe_b, np.float32), np.asarray(pe_m, np.float32),
        np.asarray(pe_v, np.float32),
        np.asarray(proj_w, np.float32), np.asarray(proj_g, np.float32),
        np.asarray(proj_b, np.float32), np.asarray(proj_m, np.float32),
        np.asarray(proj_v, np.float32))

    x = np.asarray(x, np.float32)
    in_maps = []
    for b in range(B):
        m = dict(w)
        m["x_bf"] = x[b].reshape(DIM, N).astype(BF16)
        in_maps.append(m)

    nc = _get_module()
    res = run_bass_kernel_spmd(nc, in_maps, core_ids=list(range(NCORES)),
                               trace=_trace, **(_trace_kwargs or {}))
    out = np.stack([res.results[b]["y"].reshape(DIM, 32, 32)
                    for b in range(B)])
    if _trace:
        return out.astype(np.float32), res
    return out.astype(np.float32)


# revision 34
# speedup vs baseline: 1.3753x; 1.3753x over previous
"""Trainium2 Bass kernel for nn_Attention_18940805775470.

8-sample batch of a per-sample attention block (EfficientViT-style
cascaded-group-attention cell):
  qkv 1x1 conv + BN -> 8-head attention (kd=16, hd=32, n=1024 tokens)
  -> + depthwise 3x3 BN branch on v -> 1x1 proj + BN.

Distribution: data-parallel, one sample per NeuronCore (B=8 == 8 cores).
All BN folds are done host-side; device does bf16 matmuls with fp32 PSUM
accumulation.

The kernel is softmax-bound: 8 heads x 1024^2 scores must be exp'd
(8.4M elements/core).  A single engine cannot do that fast enough, so the
exp stream is SPLIT across two engines running in parallel:
  - ScalarE: exact ACTIVATE(Exp) straight out of PSUM (~1.35us / [128,1024])
  - VectorE: Schraudolph bit-trick exp in ONE tensor_scalar op:
      i16 = rne(S * 2^7/ln2 + 16250); bitcast(i16) ~= bf16(exp(S)) +-3.5%
    (verified on HW: the DVE fp32->int16 output convert rounds to nearest;
    softmax normalization cancels most of the bias -> end-to-end rel-max
    error contribution ~2.8e-3, well inside the 2e-2 gate).
Per jc, score tile a (heads 0,1) goes to ScalarE and tile b (heads 2,3)
to VectorE; a few b tiles are reassigned to ScalarE to balance VectorE's
other elementwise work (copies / normalize).

Other structure vs the v1 kernel:
  - pe branch folded into proj: out = proj@tmp + proj@pe (+bias), PSUM
    accumulation -> the xattn adds disappear and the conv deadline moves
    to the proj at the end of each ic-half (conv units spread over all 4
    groups instead of crammed into ic=0).
  - memsets on GpSimd, off the critical VectorE.
  - scores S2[j,i] = k^T q per head, head-pairs into [128,1024] PSUM tiles
    via tile_position row tiling; softmax without max-subtraction (|S|<9).
  - ON[d,i] = v0 @ E2 and sums s[i] = 1^T E2 via 4-way column tiling.
  - qk-pack/vT/v/depthwise-conv jobs drip-fed into the PE's exp-wait slack.
"""

import sys

sys.path.insert(0, "/opt/trn_rl_repo")

import numpy as np
import ml_dtypes

BF16 = ml_dtypes.bfloat16

DIM = 256
NH = 8
HD = 32
KD = 16
SCALE = KD ** -0.5
EPS = 1e-3
B = 8
N = 1024  # 32*32 tokens
NCORES = 8
NGRP = 2  # head groups of 4

# Schraudolph constants: exp(x) ~= bitcast_bf16(int16(x * 2^7/ln2 + C))
EXP_A = float(2.0 ** 7 / np.log(2.0))
EXP_C = 16250.0

_CACHE = {}


def _build_host_weights(qkv_w, qkv_g, qkv_b, qkv_m, qkv_v,
                        pe_w, pe_g, pe_b, pe_m, pe_v,
                        proj_w, proj_g, proj_b, proj_m, proj_v):
    """Fold BN into weights and build the device-layout arrays."""
    inv_qkv = qkv_g / np.sqrt(qkv_v + EPS)
    Wq_full = qkv_w * inv_qkv[:, None]          # [512, 256]
    bq_full = qkv_b - qkv_m * inv_qkv           # [512]

    inv_pe = pe_g / np.sqrt(pe_v + EPS)
    bpe = pe_b - pe_m * inv_pe                  # [256]
    wpe = pe_w[:, 0] * inv_pe[:, None, None]    # [256, 3, 3]

    inv_p = proj_g / np.sqrt(proj_v + EPS)
    Pw = proj_w * inv_p[:, None]                # [256, 256]
    bp = proj_b - proj_m * inv_p                # [256]

    # q/k packed weight tiles: [128, NGRP*2*128]; block (g, kc) holds
    # lhsT [cc, m] with m = 32c + t.
    wq = np.zeros((128, NGRP * 2 * 128), np.float32)
    wk = np.zeros((128, NGRP * 2 * 128), np.float32)
    bqp = np.zeros((128, NGRP), np.float32)
    for g in range(NGRP):
        for c in range(4):
            h = 4 * g + c
            for kc in range(2):
                col0 = (g * 2 + kc) * 128
                # q rows (scaled); t in [0,16)
                wq[:, col0 + 32 * c: col0 + 32 * c + KD] = \
                    SCALE * Wq_full[h * 64: h * 64 + KD,
                                    kc * 128:(kc + 1) * 128].T
                # k rows, packed at the strip base like q (device reads
                # rows 32c..32c+16 of the separate kp tile)
                wk[:, col0 + 32 * c: col0 + 32 * c + KD] = \
                    Wq_full[h * 64 + KD: h * 64 + 2 * KD,
                            kc * 128:(kc + 1) * 128].T
            bqp[32 * c: 32 * c + KD, g] = \
                SCALE * bq_full[h * 64: h * 64 + KD]

    # v weights, channel-major (c = h*32 + d), transposed for lhsT/rhs use.
    vrows = np.array([(o // HD) * 64 + 2 * KD + (o % HD) for o in range(DIM)])
    Wv = Wq_full[vrows]                         # [256, 256]
    bv = bq_full[vrows]                         # [256]
    wv = np.zeros((128, 2 * 256), np.float32)   # [cc, kc*256 + o]
    for kc in range(2):
        wv[:, kc * 256:(kc + 1) * 256] = Wv[:, kc * 128:(kc + 1) * 128].T

    # depthwise conv diag tiles: [128, 2*9*128]
    dg = np.zeros((128, 2 * 9 * 128), np.float32)
    idx = np.arange(128)
    for ct in range(2):
        for tap in range(9):
            dy, dx = tap // 3, tap % 3
            blk = (ct * 9 + tap) * 128
            dg[idx, blk + idx] = wpe[ct * 128 + idx, dy, dx]

    # proj lhsT tiles: [128, (kc*2 + oc)*128 + o]
    pp = np.zeros((128, 4 * 128), np.float32)
    for kc in range(2):
        for oc in range(2):
            pp[:, (kc * 2 + oc) * 128:(kc * 2 + oc + 1) * 128] = \
                Pw[oc * 128:(oc + 1) * 128, kc * 128:(kc + 1) * 128].T

    bias_final = bp + Pw @ (bpe + bv)           # [256]

    bias_mat = np.zeros((128, 8), np.float32)
    bias_mat[:, 0:2] = bqp
    bias_mat[:, 2] = bv[:128]
    bias_mat[:, 3] = bv[128:]
    bias_mat[:, 4] = bias_final[:128]
    bias_mat[:, 5] = bias_final[128:]

    return {
        "wq": wq.astype(BF16),
        "wk": wk.astype(BF16),
        "wv": wv.astype(BF16),
        "dg": dg.astype(BF16),
        "pp": pp.astype(BF16),
        "bias": bias_mat,
    }


def _build_module():
    import concourse.bass as bass
    import concourse.mybir as mybir
    import concourse.tile as tile
    from concourse import bacc

    fp32 = mybir.dt.float32
    bf16 = mybir.dt.bfloat16
    i16 = mybir.dt.int16
    AF = mybir.ActivationFunctionType
    ALU = mybir.AluOpType

    nc = bacc.Bacc("TRN2", target_bir_lowering=False, debug=False,
                   num_devices=NCORES)

    x_d = nc.dram_tensor("x_bf", [DIM, N], bf16, kind="ExternalInput")
    wq_d = nc.dram_tensor("wq", [128, NGRP * 2 * 128], bf16, kind="ExternalInput")
    wk_d = nc.dram_tensor("wk", [128, NGRP * 2 * 128], bf16, kind="ExternalInput")
    wv_d = nc.dram_tensor("wv", [128, 2 * 256], bf16, kind="ExternalInput")
    dg_d = nc.dram_tensor("dg", [128, 2 * 9 * 128], bf16, kind="ExternalInput")
    pp_d = nc.dram_tensor("pp", [128, 4 * 128], bf16, kind="ExternalInput")
    bias_d = nc.dram_tensor("bias", [128, 8], fp32, kind="ExternalInput")
    y_d = nc.dram_tensor("y", [DIM, N], fp32, kind="ExternalOutput")

    with tile.TileContext(nc) as tc:
        from contextlib import ExitStack
        with ExitStack() as ctx:
            const = ctx.enter_context(tc.tile_pool(name="const", bufs=1))

            # ---- load inputs/weights ----
            xb = []
            for kc in range(2):
                t = const.tile([128, N], bf16, tag=f"xb{kc}", name=f"xb{kc}")
                for hh in range(2):
                    nc.sync.dma_start(
                        t[:, hh * 512:(hh + 1) * 512],
                        x_d[kc * 128:(kc + 1) * 128, hh * 512:(hh + 1) * 512])
                xb.append(t)
            wq_sb = const.tile([128, NGRP * 2 * 128], bf16, tag="wq")
            nc.gpsimd.dma_start(wq_sb[:], wq_d[:])
            wk_sb = const.tile([128, NGRP * 2 * 128], bf16, tag="wk")
            nc.gpsimd.dma_start(wk_sb[:], wk_d[:])
            bias_sb = const.tile([128, 8], fp32, tag="bias")
            nc.gpsimd.dma_start(bias_sb[:], bias_d[:])
            wv_sb = const.tile([128, 2 * 256], bf16, tag="wv")
            nc.gpsimd.dma_start(wv_sb[:], wv_d[:])
            dg_sb = const.tile([128, 2 * 9 * 128], bf16, tag="dg")
            nc.gpsimd.dma_start(dg_sb[:], dg_d[:])
            pp_sb = const.tile([128, 4 * 128], bf16, tag="pp")
            nc.gpsimd.dma_start(pp_sb[:], pp_d[:])

            ones_sb = const.tile([128, 32], bf16, tag="ones")
            nc.gpsimd.memset(ones_sb[:], 1.0)

            # ---- persistent intermediate tiles ----
            qp_sb = [const.tile([128, N], bf16, tag=f"qp{g}", name=f"qp{g}") for g in range(NGRP)]
            kp_sb = [const.tile([128, N], bf16, tag=f"kp{g}", name=f"kp{g}") for g in range(NGRP)]
            vt_sb = [const.tile([128, DIM], bf16, tag=f"vt{pc}", name=f"vt{pc}") for pc in range(8)]
            vpad = [const.tile([128, 34 * 34], bf16, tag=f"vpad{ct}", name=f"vpad{ct}") for ct in range(2)]
            pe_sb = [const.tile([128, N], bf16, tag=f"pe{ct}", name=f"pe{ct}") for ct in range(2)]
            tmp_sb = [const.tile([128, N], bf16, tag=f"tmp{ct}", name=f"tmp{ct}") for ct in range(2)]
            out_sb = [const.tile([128, N], fp32, tag=f"out{oc}", name=f"out{oc}") for oc in range(2)]

            for ct in range(2):
                nc.gpsimd.memset(vpad[ct][:], 0.0)

            # ====== unified pipeline: prologue jobs drip into attention ======
            # PSUM: scores 3x[128,1024] (6 banks) + on (1) + s (1) = 8.
            # All prologue-style matmul jobs (qk-pack, vT, v, depthwise conv)
            # borrow a scores slot briefly (tag "sc"), and are drip-fed into
            # the attention loop so the scores->exp stream starts ~immediately
            # and the PE fills its exp-wait slack with them.
            with tc.tile_pool(name="scps", bufs=3, space="PSUM") as scps, \
                 tc.tile_pool(name="onps", bufs=1, space="PSUM") as onps, \
                 tc.tile_pool(name="sps", bufs=1, space="PSUM") as sps, \
                 tc.tile_pool(name="e2", bufs=8) as e2p, \
                 tc.tile_pool(name="nrm", bufs=4) as nrm:

                def mm_ksplit(out, lhsT, rhs, first, last):
                    nc.tensor.matmul(out, lhsT, rhs, start=first, stop=last)

                def q_job(g, nc2):
                    sl = slice(nc2 * 512, (nc2 + 1) * 512)
                    pq = scps.tile([128, 512], fp32, tag="sc", name="pq")
                    for kc in range(2):
                        col = (g * 2 + kc) * 128
                        mm_ksplit(pq[:], wq_sb[:, col:col + 128],
                                  xb[kc][:, sl], kc == 0, kc == 1)
                    nc.vector.tensor_scalar_add(
                        qp_sb[g][:, sl], pq[:], bias_sb[:, g:g + 1])

                def k_job(g, nc2):
                    sl = slice(nc2 * 512, (nc2 + 1) * 512)
                    pk = scps.tile([128, 512], fp32, tag="sc", name="pk")
                    for kc in range(2):
                        col = (g * 2 + kc) * 128
                        mm_ksplit(pk[:], wk_sb[:, col:col + 128],
                                  xb[kc][:, sl], kc == 0, kc == 1)
                    nc.vector.tensor_copy(kp_sb[g][:, sl], pk[:])

                def vt_job(pc):
                    pvt = scps.tile([128, 256], fp32, tag="sc", name="pvt")
                    for kc in range(2):
                        mm_ksplit(pvt[:], xb[kc][:, pc * 128:(pc + 1) * 128],
                                  wv_sb[:, kc * 256:(kc + 1) * 256],
                                  kc == 0, kc == 1)
                    nc.vector.tensor_copy(vt_sb[pc][:], pvt[:])

                def v_job(ct, nc2):
                    vp3 = vpad[ct][:].rearrange("p (a b) -> p a b", a=34)
                    y0 = nc2 * 16
                    pv = scps.tile([128, 512], fp32, tag="sc", name="pv")
                    for kc in range(2):
                        mm_ksplit(
                            pv[:],
                            wv_sb[:, kc * 256 + ct * 128: kc * 256 + ct * 128 + 128],
                            xb[kc][:, nc2 * 512:(nc2 + 1) * 512],
                            kc == 0, kc == 1)
                    nc.vector.tensor_scalar_add(
                        vp3[:, 1 + y0:1 + y0 + 16, 1:33],
                        pv[:].rearrange("p (a b) -> p a b", b=32),
                        bias_sb[:, 2 + ct:3 + ct])

                pe_state = {}

                def pe_third(ct, nc2, t3):
                    vp3 = vpad[ct][:].rearrange("p (a b) -> p a b", a=34)
                    y0 = nc2 * 16
                    if t3 == 0:
                        pe_state[(ct, nc2)] = scps.tile(
                            [128, 512], fp32, tag="sc", name="peps")
                    pp_ps = pe_state[(ct, nc2)]
                    for tap in range(3 * t3, 3 * t3 + 3):
                        dy, dx = tap // 3, tap % 3
                        blk = (ct * 9 + tap) * 128
                        for r in range(4):
                            nc.tensor.matmul(
                                pp_ps[32 * r:32 * r + 32, :],
                                dg_sb[32 * r:32 * r + 32,
                                      blk + 32 * r:blk + 32 * r + 32],
                                vp3[32 * r:32 * r + 32,
                                    y0 + dy:y0 + dy + 16, dx:dx + 32],
                                start=(tap == 0), stop=(tap == 8),
                                tile_position=(32 * r, 32 * r),
                                skip_group_check=True)
                    if t3 == 2:
                        nc.vector.tensor_copy(
                            pe_sb[ct][:, nc2 * 512:(nc2 + 1) * 512], pp_ps[:])

                # proj split per output half and decoupled from its +bias/DMA
                # so each piece is < one exp-duration of PE work and the
                # VectorE out-add never heads the queue before its pj is done.
                pj_state = {}

                def pj_pe(ic2, oc):
                    # pe-branch half of the proj accumulation; can run as
                    # soon as the conv outputs exist (before the group's
                    # normalize produces tmp).
                    isl2 = slice(ic2 * 512, (ic2 + 1) * 512)
                    pj = scps.tile([128, 512], fp32, tag="sc", name="pj")
                    pj_state[(ic2, oc)] = pj
                    for kc in range(2):
                        col = (kc * 2 + oc) * 128
                        mm_ksplit(pj[:], pp_sb[:, col:col + 128],
                                  pe_sb[kc][:, isl2], kc == 0, False)

                def pj_tmp0(ic2, oc):
                    isl2 = slice(ic2 * 512, (ic2 + 1) * 512)
                    pj = pj_state[(ic2, oc)]
                    mm_ksplit(pj[:], pp_sb[:, oc * 128:oc * 128 + 128],
                              tmp_sb[0][:, isl2], False, False)

                def pj_tmp1(ic2, oc):
                    isl2 = slice(ic2 * 512, (ic2 + 1) * 512)
                    pj = pj_state[(ic2, oc)]
                    col = (2 + oc) * 128
                    mm_ksplit(pj[:], pp_sb[:, col:col + 128],
                              tmp_sb[1][:, isl2], False, True)

                def pj_tmp(ic2, oc):
                    pj_tmp0(ic2, oc)
                    pj_tmp1(ic2, oc)

                def pj_job(ic2, oc):
                    pj_pe(ic2, oc)
                    pj_tmp(ic2, oc)

                def out_job(ic2, oc):
                    isl2 = slice(ic2 * 512, (ic2 + 1) * 512)
                    pj = pj_state.pop((ic2, oc))
                    nc.vector.tensor_scalar_add(
                        out_sb[oc][:, isl2], pj[:], bias_sb[:, 4 + oc:5 + oc])
                    nc.sync.dma_start(
                        y_d[oc * 128:(oc + 1) * 128, isl2],
                        out_sb[oc][:, isl2])

                # prologue: only what the first scores need.
                q_job(0, 0)
                k_job(0, 0)

                # drip schedule keyed by (gi, jc); conv unit (ct, nc2) is
                # only needed by proj(ic=nc2), i.e. by the end of group 1
                # (nc2=0) / group 3 (nc2=1), so conv spreads over all groups.
                # proj(ic=0) is deferred into group 2's slots; proj(ic=1)
                # runs inline after the last group.
                drip = {
                    (0, 0): [lambda: vt_job(0), lambda: vt_job(1)],
                    (0, 1): [lambda: vt_job(2), lambda: vt_job(3),
                             lambda: k_job(0, 1)],
                    (0, 2): [lambda: vt_job(4), lambda: v_job(0, 0)],
                    (0, 3): [lambda: vt_job(5), lambda: v_job(0, 1)],
                    (0, 4): [lambda: vt_job(6), lambda: vt_job(7)],
                    (0, 5): [lambda: pe_third(0, 0, 0)],
                    (0, 6): [lambda: pe_third(0, 0, 1)],
                    (0, 7): [lambda: q_job(1, 0), lambda: k_job(1, 0)],
                    (1, 0): [lambda: pe_third(0, 0, 2), lambda: k_job(1, 1)],
                    (1, 1): [lambda: v_job(1, 0), lambda: v_job(1, 1)],
                    (1, 2): [lambda: pe_third(1, 0, 0)],
                    (1, 3): [lambda: pe_third(1, 0, 1)],
                    (1, 4): [lambda: pe_third(1, 0, 2)],
                    (1, 5): [lambda: q_job(0, 1)],
                    (2, 0): [lambda: pj_job(0, 0)],
                    (2, 1): [lambda: out_job(0, 0), lambda: pe_third(0, 1, 0)],
                    (2, 2): [lambda: pe_third(0, 1, 1)],
                    (2, 3): [lambda: pe_third(0, 1, 2), lambda: pj_job(0, 1)],
                    (2, 4): [lambda: out_job(0, 1), lambda: q_job(1, 1)],
                    (3, 0): [lambda: pe_third(1, 1, 0)],
                    (3, 1): [lambda: pe_third(1, 1, 1)],
                    (3, 2): [lambda: pe_third(1, 1, 2)],
                    (3, 6): [lambda: pj_pe(1, 0), lambda: pj_tmp0(1, 0)],
                    (3, 7): [lambda: pj_pe(1, 1), lambda: pj_tmp0(1, 1)],
                }

                # exp-engine balance: tile a (heads 0,1) -> ScalarE; tile b
                # (heads 2,3) -> VectorE Schraudolph, except EXTRA_S where b
                # also goes to ScalarE (VectorE carries the other elementwise
                # work, so ScalarE takes ~60% of the exp tiles).
                EXTRA_S = {(0, 3), (1, 3), (2, 3), (3, 3),
                           (0, 6), (1, 6), (2, 6), (3, 6)}

                def scores_one(gi, g, isl, jc, half):
                    """One [128,1024] scores tile (2 heads) + its exp.
                    half 0 (heads 0,1) -> ScalarE; half 1 -> VectorE
                    Schraudolph unless (gi,jc) in EXTRA_S."""
                    sc = scps.tile([128, 1024], fp32, tag="sc", name="sc")
                    for cc in range(2):
                        c = half * 2 + cc
                        nc.tensor.matmul(
                            sc[:, cc * 512:(cc + 1) * 512],
                            kp_sb[g][32 * c:32 * c + KD,
                                     jc * 128:(jc + 1) * 128],
                            qp_sb[g][32 * c:32 * c + KD, isl],
                            start=True, stop=True,
                            tile_position=(32 * c, 0))
                    e = e2p.tile([128, 1024], bf16, tag="e2", name="e2")
                    if half == 0 or (gi, jc) in EXTRA_S:
                        nc.scalar.activation(e[:], sc[:], AF.Exp)
                    else:
                        nc.vector.tensor_scalar(
                            e[:].bitcast(i16), sc[:], EXP_A, EXP_C,
                            ALU.mult, ALU.add)
                    return e

                # (ic, g) iteration order; the next group's jc0 scores are
                # prefetched before the previous group's vsums(7)+combine so
                # seams never stall the exp stream.
                groups = [(ic, g) for ic in range(2) for g in range(NGRP)]
                prefetched = None
                for gi, (ic, g) in enumerate(groups):
                    isl = slice(ic * 512, (ic + 1) * 512)
                    e2 = {}
                    if prefetched is not None:
                        e2[0] = prefetched
                    prefetched = None
                    on_ps = onps.tile([128, 512], fp32, tag="on", name="on")
                    s_ps = sps.tile([128, 512], fp32, tag="s", name="s")

                    def vsums(jc):
                        for c in range(4):
                            h = 4 * g + c
                            nc.tensor.matmul(
                                on_ps[32 * c:32 * c + 32, :],
                                vt_sb[jc][:, h * 32:(h + 1) * 32],
                                e2[jc][c // 2][:, (c % 2) * 512:(c % 2) * 512 + 512],
                                start=(jc == 0), stop=(jc == 7),
                                tile_position=(0, 32 * c),
                                skip_group_check=True)
                        for c in range(4):
                            nc.tensor.matmul(
                                s_ps[32 * c:32 * c + 32, :],
                                ones_sb[:],
                                e2[jc][c // 2][:, (c % 2) * 512:(c % 2) * 512 + 512],
                                start=(jc == 0), stop=(jc == 7),
                                tile_position=(0, 32 * c),
                                skip_group_check=True)

                    start = len(e2)
                    if start == 1:
                        for job in drip.get((gi, 0), []):
                            job()
                    for jc in range(start, 8):
                        e2[jc] = [scores_one(gi, g, isl, jc, 0),
                                  scores_one(gi, g, isl, jc, 1)]
                        if jc >= 1:
                            vsums(jc - 1)
                        for job in drip.get((gi, jc), []):
                            job()
                    if gi + 1 < len(groups):
                        nic, ng = groups[gi + 1]
                        nisl = slice(nic * 512, (nic + 1) * 512)
                        prefetched = [
                            scores_one(gi + 1, ng, nisl, 0, 0),
                            scores_one(gi + 1, ng, nisl, 0, 1)]
                    vsums(7)
                    rbc = nrm.tile([128, 512], fp32, tag="rbc", name="rbc")
                    nc.vector.reciprocal_approx_fast(rbc[:], s_ps[:])
                    nc.vector.tensor_mul(tmp_sb[g][:, isl], on_ps[:], rbc[:])

                # tail: proj(ic=1) could not be deferred into a later group.
                # pe-halves and the g=0 tmp-half ran in drip slots; only the
                # g=1 tmp matmuls sit behind the last normalize.
                pj_tmp1(1, 0)
                pj_tmp1(1, 1)
                out_job(1, 0)
                out_job(1, 1)

    nc.compile()
    return nc


def _get_module():
    if "nc" not in _CACHE:
        _CACHE["nc"] = _build_module()
    return _CACHE["nc"]


def kernel(x, qkv_w, qkv_g, qkv_b, qkv_m, qkv_v,
           pe_w, pe_g, pe_b, pe_m, pe_v,
           proj_w, proj_g, proj_b, proj_m, proj_v,
           _trace=False, _trace_kwargs=None):
    from concourse.bass_utils import run_bass_kernel_spmd

    w = _build_host_weights(
        np.asarray(qkv_w, np.float32), np.asarray(qkv_g, np.float32),
        np.asarray(qkv_b, np.float32), np.asarray(qkv_m, np.float32),
        np.asarray(qkv_v, np.float32),
        np.asarray(pe_w, np.float32), np.asarray(pe_g, np.float32),
        np.asarray(pe_b, np.float32), np.asarray(pe_m, np.float32),
        np.asarray(pe_v, np.float32),
        np.asarray(proj_w, np.float32), np.asarray(proj_g, np.float32),
        np.asarray(proj_b, np.float32), np.asarray(proj_m, np.float32),
        np.asarray(proj_v, np.float32))

    x = np.asarray(x, np.float32)
    in_maps = []
    for b in range(B):
        m = dict(w)
        m["x_bf"] = x[b].reshape(DIM, N).astype(BF16)
        in_maps.append(m)

    nc = _get_module()
    res = run_bass_kernel_spmd(nc, in_maps, core_ids=list(range(NCORES)),
                               trace=_trace, **(_trace_kwargs or {}))
    out = np.stack([res.results[b]["y"].reshape(DIM, 32, 32)
                    for b in range(B)])
    if _trace:
        return out.astype(np.float32), res
    return out.astype(np.float32)
